# revision 37
# baseline (speedup 1.0000x reference)
"""Trainium2 Bass kernel for nn_BackgroundStd2D.

Computes, per (batch, channel): the unbiased std of bf over the pixels where
aspp_mask <= 0.5, clamped below by min_std + 1e-6.

Sharding: pure data parallel over the 1024 (batch, channel) rows of
bf.reshape(B*C, H*W); each of the 8 cores handles 128 rows (one batch's
half of channels) plus that batch's mask.

Per-core algorithm (rows on partitions, pixels on the free axis):
  keep128[p, f] = (mask[p*512+f] <= 0.5) in bf16 (exact 0/1)  [128, 512]
  keep is re-laid out to [4, n_chunks, 512] via a DRAM bounce; per 2048-px
  chunk the PE broadcasts the 4 keep rows across all 128 partitions into
  PSUM with one-hot bf16 selector matmuls (bf16 keeps PE at full rate).
  DVE scalar_tensor_tensor (stock ISA): bfk = (bf*1)*keep written in place
  over the bf tile, accum_out = s_part (fused multiply+sum, one pass).
  ACT activation(Square, accum_out): q_part = sum(bfk^2), second pass.
  n = sum(keep) via free-axis reduce + DRAM-bounce partition fold.
  Final [128,1] math: std = sqrt(q/(n-1) - s^2/(n(n-1))), out = max(std,
  min_std + 1e-6). All n-only terms (reciprocals, clamp floor) are emitted
  BEFORE the chunk loop so the in-order engine queues run them during the
  stream; ACT sqrt alone is accurate to ~4e-7 rel here, so no Newton steps.

  Steady state is HBM-bound: ~99-101us/pass vs ~96-98us pure-DMA floor
  (32 MiB/core at ~350 GB/s, 97% of the 358 GB/s HBM-per-NC limit; the
  floor drifts ~1-2us with ambient load on the shared device). The ~3.3us
  gap over the floor is per-consumer-stage interference (PE/DVE/ACT each
  add ~1us/pass) and is invariant to operand bytes (bf16/fp8 variants),
  PSUM-residency, issue rings, chunk size (2048 px optimal both ways),
  and buffer depth - measured exhaustively; see sweep.py/duel.py.
"""

import sys

sys.path.insert(0, "/opt/trn_rl_repo")

import numpy as np

import concourse.bass as bass
import concourse.tile as tile
from concourse import bacc, mybir
from concourse.bass_utils import run_bass_kernel_spmd

P = 128
N_CORES = 8
MIN_STD_VAL = 1e-05

F32 = mybir.dt.float32
BF16 = mybir.dt.bfloat16
ALU = mybir.AluOpType
ACTF = mybir.ActivationFunctionType


def build_bass(
    npix: int,
    dma_chunk: int = 4096,
    ttr_chunk: int = 2048,
    iters: int = 1,
    mode: str = "full",  # full | noact | nostt | dmaonly | dmapure
    rings: str = "sync",  # sync | dual | tri
    bf_bufs: int = 3,
    hw_loop: bool = False,
    dual_ring: bool = False,
    in_place: bool = False,
    pixmaj: bool = False,
    psum_bfk: bool = False,
    newton: int = 2,
    kp_bufs: int = 2,
    blk: int = 512,
    bfk_dt: str = "f32",
    bfk_bufs: int | None = None,
    dve_q: bool = False,
    keep_dt: str = "bf16",
    q_eng: str = "act",  # act | dve | gp
) -> bass.Bass:
    assert npix % dma_chunk == 0 and dma_chunk % ttr_chunk == 0
    assert ttr_chunk % blk == 0
    n_blk = npix // blk  # blk-pixel blocks; one keep row per block
    assert n_blk <= P
    n_dma = npix // dma_chunk
    n_ttr = npix // ttr_chunk
    BFK = {"f32": F32, "bf16": BF16}[bfk_dt]
    KDT = {"bf16": BF16, "fp8": mybir.dt.float8e4}[keep_dt]
    if bfk_bufs is None:
        bfk_bufs = bf_bufs

    nc = bacc.Bacc("TRN2", target_bir_lowering=False, debug=False)

    if pixmaj:
        bf_d = nc.dram_tensor(
            "bf", [npix // dma_chunk, P, dma_chunk], F32, kind="ExternalInput"
        ).ap()
    else:
        bf_d = nc.dram_tensor("bf", [P, npix], F32, kind="ExternalInput").ap()
    mask_d = nc.dram_tensor("mask", [n_blk, blk], F32, kind="ExternalInput").ap()
    mins_d = nc.dram_tensor("min_std", [P, 1], F32, kind="ExternalInput").ap()
    out_d = nc.dram_tensor("out", [P, 1], F32, kind="ExternalOutput").ap()
    keep_scratch = nc.dram_tensor("keep_scratch", [npix], KDT).ap()
    cnt_scratch = nc.dram_tensor("cnt_scratch", [P], F32).ap()
    n_scratch = nc.dram_tensor("n_scratch", [1], F32).ap()
    nsel = ttr_chunk // blk
    sel_d = nc.dram_tensor("sels", [nsel, nsel, P], KDT, kind="ExternalInput").ap()

    with tile.TileContext(nc) as tc:
        with (
            tc.tile_pool(name="singles", bufs=1) as singles,
            tc.tile_pool(name="bfp", bufs=bf_bufs) as bf_pool,
            tc.tile_pool(name="bfkp", bufs=bfk_bufs) as bfk_pool,
            tc.tile_pool(name="kps", bufs=kp_bufs, space="PSUM") as kp_pool,
            tc.tile_pool(name="fin", bufs=2) as fin,
        ):
            # One-hot row selectors: sel[k].T @ keep_r[:, J, :] broadcasts
            # keep row k across all 128 output partitions.
            sel_t = singles.tile([nsel, nsel, P], KDT)
            nc.scalar.dma_start(out=sel_t, in_=sel_d)
            sels = [sel_t[:, k, :] for k in range(nsel)]

            mask128 = singles.tile([n_blk, blk], F32)
            nc.scalar.dma_start(out=mask128, in_=mask_d)
            # keep is exactly 0/1 so bf16 is lossless; bf16 operands keep the
            # PE broadcast matmuls at full (non-fp32) rate.
            keep128 = singles.tile([n_blk, blk], KDT)
            nc.vector.tensor_scalar(
                out=keep128, in0=mask128, scalar1=0.5, scalar2=None, op0=ALU.is_le
            )
            # Bounce through DRAM to land keep in [4, n_ttr, 512] layout:
            # partition a holds pixel blocks {4m + a}.
            nc.scalar.dma_start(out=keep_scratch, in_=keep128)
            keep_r = singles.tile([nsel, n_ttr, blk], KDT)
            nc.scalar.dma_start(
                out=keep_r,
                in_=keep_scratch.rearrange("(m a f) -> a m f", a=nsel, f=blk),
            )

            # n = sum(keep): free-axis reduce, then fold the 128 partition
            # partials onto one partition via a DRAM bounce, reduce, and
            # broadcast the scalar back to all partitions.
            cnt = singles.tile([P, 1], F32)
            nc.vector.memset(cnt, 0.0)
            nc.vector.reduce_sum(
                out=cnt[0:n_blk, :], in_=keep128, axis=mybir.AxisListType.X
            )
            nc.scalar.dma_start(out=cnt_scratch, in_=cnt)
            cnt_row = singles.tile([1, P], F32)
            nc.scalar.dma_start(out=cnt_row, in_=cnt_scratch)
            n_scalar = singles.tile([1, 1], F32)
            nc.vector.reduce_sum(out=n_scalar, in_=cnt_row, axis=mybir.AxisListType.X)
            nc.scalar.dma_start(out=n_scratch, in_=n_scalar)
            n_b = singles.tile([P, 1], F32)
            nc.scalar.dma_start(out=n_b, in_=n_scratch.to_broadcast([P, 1]))

            minstd_sb = singles.tile([P, 1], F32)
            nc.scalar.dma_start(out=minstd_sb, in_=mins_d)

            # n-only final-math terms, emitted BEFORE the loop: engine queues
            # are in-order, so anything emitted after the loop only runs once
            # the last chunk drains. var = q/(n-1) - s^2*(1/(n(n-1))).
            inv_n = singles.tile([P, 1], F32)
            nc.vector.reciprocal(inv_n, n_b)
            nm1 = singles.tile([P, 1], F32)
            nc.vector.tensor_scalar_add(nm1, n_b, -1.0)
            inv_nm1 = singles.tile([P, 1], F32)
            nc.vector.reciprocal(inv_nm1, nm1)
            c2 = singles.tile([P, 1], F32)
            nc.vector.tensor_mul(c2, inv_n, inv_nm1)
            lower = singles.tile([P, 1], F32)
            nc.vector.tensor_scalar_add(lower, minstd_sb, MIN_STD_VAL / 10.0)

            s_parts = singles.tile([P, n_ttr], F32)
            q_parts = singles.tile([P, n_ttr if psum_bfk else n_dma], F32)
            if mode != "full":
                nc.vector.memset(q_parts, 1.0)
                nc.vector.memset(s_parts, 1.0)

            import contextlib

            loop_cm = (
                tc.For_i(0, iters, 1) if hw_loop else contextlib.nullcontext(range(iters))
            )
            with loop_cm as _loop:
              for _it in range(1 if hw_loop else iters):
               for c in range(n_dma):
                bf_t = bf_pool.tile([P, dma_chunk], F32)
                if dual_ring or rings == "dual":
                    dma_eng = (nc.sync, nc.scalar)[c % 2]
                elif rings == "tri":
                    dma_eng = (nc.sync, nc.scalar, nc.gpsimd)[c % 3]
                else:
                    dma_eng = nc.sync
                bf_src = bf_d[c] if pixmaj else bf_d[:, c * dma_chunk : (c + 1) * dma_chunk]
                dma_eng.dma_start(out=bf_t, in_=bf_src)
                if mode == "dmapure":
                    continue
                bfk_t = (
                    bf_t
                    if (in_place or psum_bfk)
                    else bfk_pool.tile([P, dma_chunk], BFK)
                )
                if mode == "dmaonly":
                    nc.vector.reduce_sum(
                        out=s_parts[:, c : c + 1],
                        in_=bf_t[:, 0:8],
                        axis=mybir.AxisListType.X,
                    )
                    continue
                for h in range(dma_chunk // ttr_chunk):
                    j = c * (dma_chunk // ttr_chunk) + h
                    kp = kp_pool.tile([P, ttr_chunk], F32)
                    for k in range(ttr_chunk // blk):
                        nc.tensor.matmul(
                            kp[:, blk * k : blk * (k + 1)],
                            sels[k],
                            keep_r[:, j, :],
                            start=True,
                            stop=True,
                        )
                    if mode == "nostt":
                        nc.vector.reduce_sum(
                            out=s_parts[:, j : j + 1],
                            in_=kp[:, 0:8],
                            axis=mybir.AxisListType.X,
                        )
                    elif psum_bfk:
                        # bfk lives in PSUM, in place over kp: SBUF sees only
                        # the bf DMA write + the DVE read of bf.
                        nc.vector.scalar_tensor_tensor(
                            out=kp,
                            in0=bf_t[:, h * ttr_chunk : (h + 1) * ttr_chunk],
                            scalar=1.0,
                            in1=kp,
                            op0=ALU.mult,
                            op1=ALU.mult,
                            accum_out=s_parts[:, j : j + 1],
                        )
                        if mode == "full":
                            nc.scalar.activation(
                                out=kp,
                                in_=kp,
                                func=ACTF.Square,
                                accum_out=q_parts[:, j : j + 1],
                            )
                    else:
                        nc.vector.scalar_tensor_tensor(
                            out=bfk_t[:, h * ttr_chunk : (h + 1) * ttr_chunk],
                            in0=bf_t[:, h * ttr_chunk : (h + 1) * ttr_chunk],
                            scalar=1.0,
                            in1=kp,
                            op0=ALU.mult,
                            op1=ALU.mult,
                            accum_out=s_parts[:, j : j + 1],
                        )
                if mode == "full" and not psum_bfk:
                    if dve_q or q_eng == "dve":
                        nc.vector.scalar_tensor_tensor(
                            out=bfk_t,
                            in0=bfk_t,
                            scalar=1.0,
                            in1=bfk_t,
                            op0=ALU.mult,
                            op1=ALU.mult,
                            accum_out=q_parts[:, c : c + 1],
                        )
                    elif q_eng == "gp":
                        nc.gpsimd.scalar_tensor_tensor(
                            out=bfk_t,
                            in0=bfk_t,
                            scalar=1.0,
                            in1=bfk_t,
                            op0=ALU.mult,
                            op1=ALU.mult,
                            accum_out=q_parts[:, c : c + 1],
                        )
                    else:
                        nc.scalar.activation(
                            out=bfk_t,
                            in_=bfk_t,
                            func=ACTF.Square,
                            accum_out=q_parts[:, c : c + 1],
                        )

            s = fin.tile([P, 1], F32)
            nc.vector.reduce_sum(out=s, in_=s_parts, axis=mybir.AxisListType.X)
            q = fin.tile([P, 1], F32)
            nc.vector.reduce_sum(out=q, in_=q_parts, axis=mybir.AxisListType.X)

            t1 = fin.tile([P, 1], F32)
            nc.vector.scalar_tensor_tensor(
                out=t1, in0=s, scalar=c2, in1=s, op0=ALU.mult, op1=ALU.mult
            )
            var = fin.tile([P, 1], F32)
            nc.vector.scalar_tensor_tensor(
                out=var, in0=q, scalar=inv_nm1, in1=t1, op0=ALU.mult, op1=ALU.subtract
            )

            std = fin.tile([P, 1], F32)
            nc.scalar.sqrt(std, var)
            # ACT sqrt has a loose ULP budget; Newton steps pin it to f32.
            for it in range(newton):
                r = fin.tile([P, 1], F32, name=f"r{it}")
                nc.vector.reciprocal(r, std)
                t = fin.tile([P, 1], F32, name=f"t{it}")
                nc.vector.tensor_mul(t, var, r)
                u = fin.tile([P, 1], F32, name=f"u{it}")
                nc.vector.tensor_add(u, std, t)
                std = fin.tile([P, 1], F32, name=f"std{it}")
                nc.vector.tensor_scalar_mul(std, u, 0.5)

            outv = fin.tile([P, 1], F32)
            nc.vector.tensor_max(outv, std, lower)
            nc.sync.dma_start(out=out_d, in_=outv)

    nc.compile()
    return nc


_NC_CACHE: dict[tuple, bass.Bass] = {}


def _get_nc(npix: int, **kwargs) -> bass.Bass:
    key = (npix, tuple(sorted(kwargs.items())))
    if key not in _NC_CACHE:
        _NC_CACHE[key] = build_bass(npix, **kwargs)
    return _NC_CACHE[key]


def make_in_maps(
    bf: np.ndarray,
    aspp_mask: np.ndarray,
    min_std: np.ndarray,
    pixmaj: bool = False,
    dma_chunk: int = 4096,
    ttr_chunk: int = 2048,
    blk: int = 512,
    keep_dt: str = "bf16",
):
    B, C, H, W = bf.shape
    npix = H * W
    rows = bf.reshape(B * C, npix)
    rows_per_core = (B * C) // N_CORES
    cores_per_batch = C // rows_per_core
    mask_flat = np.ascontiguousarray(aspp_mask.reshape(B, npix))
    minstd_flat = np.ascontiguousarray(min_std.reshape(C))
    sels = make_sels(ttr_chunk // blk, keep_dt)
    in_maps = []
    for k in range(N_CORES):
        b = k // cores_per_batch
        c0 = (k % cores_per_batch) * rows_per_core
        shard = rows[k * rows_per_core : (k + 1) * rows_per_core]
        if pixmaj:
            # [n_dma, P, dma_chunk]: each chunk contiguous in DRAM
            shard = np.ascontiguousarray(
                shard.reshape(P, npix // dma_chunk, dma_chunk).transpose(1, 0, 2)
            )
        else:
            shard = np.ascontiguousarray(shard)
        in_maps.append(
            {
                "bf": shard,
                "mask": mask_flat[b].reshape(npix // blk, blk),
                "min_std": minstd_flat[c0 : c0 + rows_per_core].reshape(P, 1),
                "sels": sels,
            }
        )
    return in_maps


def make_sels(nsel: int = 4, keep_dt: str = "bf16") -> np.ndarray:
    import ml_dtypes

    np_dt = {"bf16": ml_dtypes.bfloat16, "fp8": mybir.dt.np(mybir.dt.float8e4)}[
        keep_dt
    ]
    sels = np.zeros((nsel, nsel, P), dtype=np_dt)
    for k in range(nsel):
        sels[k, k, :] = 1.0
    return sels


def kernel(bf: np.ndarray, aspp_mask: np.ndarray, min_std: np.ndarray, **run_kwargs):
    bf = np.asarray(bf, dtype=np.float32)
    aspp_mask = np.asarray(aspp_mask, dtype=np.float32)
    min_std = np.asarray(min_std, dtype=np.float32)
    B, C, H, W = bf.shape
    npix = H * W

    nc = _get_nc(npix, dma_chunk=2048, bf_bufs=16, in_place=True, newton=0)
    in_maps = make_in_maps(bf, aspp_mask, min_std)
    res = run_bass_kernel_spmd(nc, in_maps, list(range(N_CORES)), **run_kwargs)

    out = np.empty((B, C), dtype=np.float32)
    rows_per_core = (B * C) // N_CORES
    cores_per_batch = C // rows_per_core
    for k in range(N_CORES):
        b = k // cores_per_batch
        c0 = (k % cores_per_batch) * rows_per_core
        out[b, c0 : c0 + rows_per_core] = res.results[k]["out"].reshape(rows_per_core)
    if run_kwargs:
        return out.reshape(B, C, 1, 1), res
    return out.reshape(B, C, 1, 1)



# revision 41
# speedup vs baseline: 1.0018x; 1.0018x over previous
"""Trainium2 Bass kernel for nn_BackgroundStd2D.

Computes, per (batch, channel): the unbiased std of bf over the pixels where
aspp_mask <= 0.5, clamped below by min_std + 1e-6.

Sharding: pure data parallel over the 1024 (batch, channel) rows of
bf.reshape(B*C, H*W); each of the 8 cores handles 128 rows (one batch's
half of channels) plus that batch's mask.

Per-core algorithm (rows on partitions, pixels on the free axis):
  keep128[p, f] = (mask[p*512+f] <= 0.5) in bf16 (exact 0/1)  [128, 512]
  keep is re-laid out to [4, n_chunks, 512] via a DRAM bounce; per 2048-px
  chunk the PE broadcasts the 4 keep rows across all 128 partitions into
  PSUM with one-hot bf16 selector matmuls (bf16 keeps PE at full rate).
  DVE scalar_tensor_tensor (stock ISA): bfk = (bf*1)*keep written in place
  over the bf tile, accum_out = s_part (fused multiply+sum, one pass).
  ACT activation(Square, accum_out): q_part = sum(bfk^2), second pass.
  n = sum(keep) via free-axis reduce + DRAM-bounce partition fold.
  Final [128,1] math: std = sqrt(q/(n-1) - s^2/(n(n-1))), out = max(std,
  min_std + 1e-6). All n-only terms (reciprocals, clamp floor) are emitted
  BEFORE the chunk loop so the in-order engine queues run them during the
  stream; ACT sqrt alone is accurate to ~4e-7 rel here, so no Newton steps.

  Steady state is HBM-bound: ~99-101us/pass vs ~96-98us pure-DMA floor
  (32 MiB/core at ~350 GB/s, 97% of the 358 GB/s HBM-per-NC limit; the
  floor drifts ~1-2us with ambient load on the shared device). The ~3.3us
  gap over the floor is per-consumer-stage interference (PE/DVE/ACT each
  add ~1us/pass) and is invariant to operand bytes (bf16/fp8 variants),
  PSUM-residency, issue rings, chunk size (2048 px optimal both ways),
  and buffer depth - measured exhaustively; see sweep.py/duel.py.
"""

import sys

sys.path.insert(0, "/opt/trn_rl_repo")

import numpy as np

import concourse.bass as bass
import concourse.tile as tile
from concourse import bacc, mybir
from concourse.bass_utils import run_bass_kernel_spmd

P = 128
N_CORES = 8
MIN_STD_VAL = 1e-05

F32 = mybir.dt.float32
BF16 = mybir.dt.bfloat16
ALU = mybir.AluOpType
ACTF = mybir.ActivationFunctionType


def build_bass(
    npix: int,
    dma_chunk: int = 4096,
    ttr_chunk: int = 2048,
    iters: int = 1,
    mode: str = "full",  # full | noact | nostt | dmaonly | dmapure
    rings: str = "sync",  # sync | dual | tri
    bf_bufs: int = 3,
    hw_loop: bool = False,
    dual_ring: bool = False,
    in_place: bool = False,
    pixmaj: bool = False,
    psum_bfk: bool = False,
    newton: int = 2,
    kp_bufs: int = 2,
    blk: int = 512,
    bfk_dt: str = "f32",
    bfk_bufs: int | None = None,
    dve_q: bool = False,
    keep_dt: str = "bf16",
    q_eng: str = "act",  # act | dve | gp
    wide_act: bool = False,  # square 2 dma chunks per ACT op
) -> bass.Bass:
    assert npix % dma_chunk == 0 and dma_chunk % ttr_chunk == 0
    assert ttr_chunk % blk == 0
    n_blk = npix // blk  # blk-pixel blocks; one keep row per block
    assert n_blk <= P
    n_dma = npix // dma_chunk
    n_ttr = npix // ttr_chunk
    BFK = {"f32": F32, "bf16": BF16}[bfk_dt]
    KDT = {"bf16": BF16, "fp8": mybir.dt.float8e4}[keep_dt]
    if bfk_bufs is None:
        bfk_bufs = bf_bufs

    nc = bacc.Bacc("TRN2", target_bir_lowering=False, debug=False)

    if pixmaj:
        bf_d = nc.dram_tensor(
            "bf", [npix // dma_chunk, P, dma_chunk], F32, kind="ExternalInput"
        ).ap()
    else:
        bf_d = nc.dram_tensor("bf", [P, npix], F32, kind="ExternalInput").ap()
    mask_d = nc.dram_tensor("mask", [n_blk, blk], F32, kind="ExternalInput").ap()
    mins_d = nc.dram_tensor("min_std", [P, 1], F32, kind="ExternalInput").ap()
    out_d = nc.dram_tensor("out", [P, 1], F32, kind="ExternalOutput").ap()
    keep_scratch = nc.dram_tensor("keep_scratch", [npix], KDT).ap()
    cnt_scratch = nc.dram_tensor("cnt_scratch", [P], F32).ap()
    n_scratch = nc.dram_tensor("n_scratch", [1], F32).ap()
    nsel = ttr_chunk // blk
    sel_d = nc.dram_tensor("sels", [nsel, nsel, P], KDT, kind="ExternalInput").ap()

    with tile.TileContext(nc) as tc:
        with (
            tc.tile_pool(name="singles", bufs=1) as singles,
            tc.tile_pool(name="bfp", bufs=bf_bufs) as bf_pool,
            tc.tile_pool(name="bfkp", bufs=bfk_bufs) as bfk_pool,
            tc.tile_pool(name="kps", bufs=kp_bufs, space="PSUM") as kp_pool,
            tc.tile_pool(name="fin", bufs=2) as fin,
        ):
            # One-hot row selectors: sel[k].T @ keep_r[:, J, :] broadcasts
            # keep row k across all 128 output partitions.
            sel_t = singles.tile([nsel, nsel, P], KDT)
            nc.scalar.dma_start(out=sel_t, in_=sel_d)
            sels = [sel_t[:, k, :] for k in range(nsel)]

            mask128 = singles.tile([n_blk, blk], F32)
            nc.scalar.dma_start(out=mask128, in_=mask_d)
            # keep is exactly 0/1 so bf16 is lossless; bf16 operands keep the
            # PE broadcast matmuls at full (non-fp32) rate.
            keep128 = singles.tile([n_blk, blk], KDT)
            nc.vector.tensor_scalar(
                out=keep128, in0=mask128, scalar1=0.5, scalar2=None, op0=ALU.is_le
            )
            # Bounce through DRAM to land keep in [4, n_ttr, 512] layout:
            # partition a holds pixel blocks {4m + a}.
            nc.scalar.dma_start(out=keep_scratch, in_=keep128)
            keep_r = singles.tile([nsel, n_ttr, blk], KDT)
            nc.scalar.dma_start(
                out=keep_r,
                in_=keep_scratch.rearrange("(m a f) -> a m f", a=nsel, f=blk),
            )

            # n = sum(keep): free-axis reduce, then fold the 128 partition
            # partials onto one partition via a DRAM bounce, reduce, and
            # broadcast the scalar back to all partitions.
            cnt = singles.tile([P, 1], F32)
            nc.vector.memset(cnt, 0.0)
            nc.vector.reduce_sum(
                out=cnt[0:n_blk, :], in_=keep128, axis=mybir.AxisListType.X
            )
            nc.scalar.dma_start(out=cnt_scratch, in_=cnt)
            cnt_row = singles.tile([1, P], F32)
            nc.scalar.dma_start(out=cnt_row, in_=cnt_scratch)
            n_scalar = singles.tile([1, 1], F32)
            nc.vector.reduce_sum(out=n_scalar, in_=cnt_row, axis=mybir.AxisListType.X)
            nc.scalar.dma_start(out=n_scratch, in_=n_scalar)
            n_b = singles.tile([P, 1], F32)
            nc.scalar.dma_start(out=n_b, in_=n_scratch.to_broadcast([P, 1]))

            minstd_sb = singles.tile([P, 1], F32)
            nc.scalar.dma_start(out=minstd_sb, in_=mins_d)

            # n-only final-math terms, emitted BEFORE the loop: engine queues
            # are in-order, so anything emitted after the loop only runs once
            # the last chunk drains. var = q/(n-1) - s^2*(1/(n(n-1))).
            inv_n = singles.tile([P, 1], F32)
            nc.vector.reciprocal(inv_n, n_b)
            nm1 = singles.tile([P, 1], F32)
            nc.vector.tensor_scalar_add(nm1, n_b, -1.0)
            inv_nm1 = singles.tile([P, 1], F32)
            nc.vector.reciprocal(inv_nm1, nm1)
            c2 = singles.tile([P, 1], F32)
            nc.vector.tensor_mul(c2, inv_n, inv_nm1)
            lower = singles.tile([P, 1], F32)
            nc.vector.tensor_scalar_add(lower, minstd_sb, MIN_STD_VAL / 10.0)

            s_parts = singles.tile([P, n_ttr], F32)
            q_parts = singles.tile(
                [P, n_ttr if psum_bfk else (n_dma // 2 if wide_act else n_dma)], F32
            )
            if mode != "full":
                nc.vector.memset(q_parts, 1.0)
                nc.vector.memset(s_parts, 1.0)

            import contextlib

            loop_cm = (
                tc.For_i(0, iters, 1) if hw_loop else contextlib.nullcontext(range(iters))
            )
            with loop_cm as _loop:
              for _it in range(1 if hw_loop else iters):
               for c in range(n_dma):
                bf_t = bf_pool.tile([P, dma_chunk], F32)
                if dual_ring or rings == "dual":
                    dma_eng = (nc.sync, nc.scalar)[c % 2]
                elif rings == "tri":
                    dma_eng = (nc.sync, nc.scalar, nc.gpsimd)[c % 3]
                else:
                    dma_eng = nc.sync
                bf_src = bf_d[c] if pixmaj else bf_d[:, c * dma_chunk : (c + 1) * dma_chunk]
                dma_eng.dma_start(out=bf_t, in_=bf_src)
                if mode == "dmapure":
                    continue
                if wide_act:
                    if c % 2 == 0:
                        bfk_wide = bfk_pool.tile([P, 2 * dma_chunk], BFK)
                    bfk_t = bfk_wide[:, (c % 2) * dma_chunk : (c % 2 + 1) * dma_chunk]
                else:
                    bfk_t = (
                        bf_t
                        if (in_place or psum_bfk)
                        else bfk_pool.tile([P, dma_chunk], BFK)
                    )
                if mode == "dmaonly":
                    nc.vector.reduce_sum(
                        out=s_parts[:, c : c + 1],
                        in_=bf_t[:, 0:8],
                        axis=mybir.AxisListType.X,
                    )
                    continue
                for h in range(dma_chunk // ttr_chunk):
                    j = c * (dma_chunk // ttr_chunk) + h
                    kp = kp_pool.tile([P, ttr_chunk], F32)
                    for k in range(ttr_chunk // blk):
                        nc.tensor.matmul(
                            kp[:, blk * k : blk * (k + 1)],
                            sels[k],
                            keep_r[:, j, :],
                            start=True,
                            stop=True,
                        )
                    if mode == "nostt":
                        nc.vector.reduce_sum(
                            out=s_parts[:, j : j + 1],
                            in_=kp[:, 0:8],
                            axis=mybir.AxisListType.X,
                        )
                    elif psum_bfk:
                        # bfk lives in PSUM, in place over kp: SBUF sees only
                        # the bf DMA write + the DVE read of bf.
                        nc.vector.scalar_tensor_tensor(
                            out=kp,
                            in0=bf_t[:, h * ttr_chunk : (h + 1) * ttr_chunk],
                            scalar=1.0,
                            in1=kp,
                            op0=ALU.mult,
                            op1=ALU.mult,
                            accum_out=s_parts[:, j : j + 1],
                        )
                        if mode == "full":
                            nc.scalar.activation(
                                out=kp,
                                in_=kp,
                                func=ACTF.Square,
                                accum_out=q_parts[:, j : j + 1],
                            )
                    else:
                        nc.vector.scalar_tensor_tensor(
                            out=bfk_t[:, h * ttr_chunk : (h + 1) * ttr_chunk],
                            in0=bf_t[:, h * ttr_chunk : (h + 1) * ttr_chunk],
                            scalar=1.0,
                            in1=kp,
                            op0=ALU.mult,
                            op1=ALU.mult,
                            accum_out=s_parts[:, j : j + 1],
                        )
                if mode == "full" and wide_act:
                    if c % 2 == 1:
                        nc.scalar.activation(
                            out=bfk_wide,
                            in_=bfk_wide,
                            func=ACTF.Square,
                            accum_out=q_parts[:, c // 2 : c // 2 + 1],
                        )
                elif mode == "full" and not psum_bfk:
                    if dve_q or q_eng == "dve":
                        nc.vector.scalar_tensor_tensor(
                            out=bfk_t,
                            in0=bfk_t,
                            scalar=1.0,
                            in1=bfk_t,
                            op0=ALU.mult,
                            op1=ALU.mult,
                            accum_out=q_parts[:, c : c + 1],
                        )
                    elif q_eng == "gp":
                        nc.gpsimd.scalar_tensor_tensor(
                            out=bfk_t,
                            in0=bfk_t,
                            scalar=1.0,
                            in1=bfk_t,
                            op0=ALU.mult,
                            op1=ALU.mult,
                            accum_out=q_parts[:, c : c + 1],
                        )
                    else:
                        nc.scalar.activation(
                            out=bfk_t,
                            in_=bfk_t,
                            func=ACTF.Square,
                            accum_out=q_parts[:, c : c + 1],
                        )

            s = fin.tile([P, 1], F32)
            nc.vector.reduce_sum(out=s, in_=s_parts, axis=mybir.AxisListType.X)
            q = fin.tile([P, 1], F32)
            nc.vector.reduce_sum(out=q, in_=q_parts, axis=mybir.AxisListType.X)

            t1 = fin.tile([P, 1], F32)
            nc.vector.scalar_tensor_tensor(
                out=t1, in0=s, scalar=c2, in1=s, op0=ALU.mult, op1=ALU.mult
            )
            var = fin.tile([P, 1], F32)
            nc.vector.scalar_tensor_tensor(
                out=var, in0=q, scalar=inv_nm1, in1=t1, op0=ALU.mult, op1=ALU.subtract
            )

            std = fin.tile([P, 1], F32)
            nc.scalar.sqrt(std, var)
            # ACT sqrt has a loose ULP budget; Newton steps pin it to f32.
            for it in range(newton):
                r = fin.tile([P, 1], F32, name=f"r{it}")
                nc.vector.reciprocal(r, std)
                t = fin.tile([P, 1], F32, name=f"t{it}")
                nc.vector.tensor_mul(t, var, r)
                u = fin.tile([P, 1], F32, name=f"u{it}")
                nc.vector.tensor_add(u, std, t)
                std = fin.tile([P, 1], F32, name=f"std{it}")
                nc.vector.tensor_scalar_mul(std, u, 0.5)

            outv = fin.tile([P, 1], F32)
            nc.vector.tensor_max(outv, std, lower)
            nc.sync.dma_start(out=out_d, in_=outv)

    nc.compile()
    return nc


_NC_CACHE: dict[tuple, bass.Bass] = {}


def _get_nc(npix: int, **kwargs) -> bass.Bass:
    key = (npix, tuple(sorted(kwargs.items())))
    if key not in _NC_CACHE:
        _NC_CACHE[key] = build_bass(npix, **kwargs)
    return _NC_CACHE[key]


def make_in_maps(
    bf: np.ndarray,
    aspp_mask: np.ndarray,
    min_std: np.ndarray,
    pixmaj: bool = False,
    dma_chunk: int = 4096,
    ttr_chunk: int = 2048,
    blk: int = 512,
    keep_dt: str = "bf16",
):
    B, C, H, W = bf.shape
    npix = H * W
    rows = bf.reshape(B * C, npix)
    rows_per_core = (B * C) // N_CORES
    cores_per_batch = C // rows_per_core
    mask_flat = np.ascontiguousarray(aspp_mask.reshape(B, npix))
    minstd_flat = np.ascontiguousarray(min_std.reshape(C))
    sels = make_sels(ttr_chunk // blk, keep_dt)
    in_maps = []
    for k in range(N_CORES):
        b = k // cores_per_batch
        c0 = (k % cores_per_batch) * rows_per_core
        shard = rows[k * rows_per_core : (k + 1) * rows_per_core]
        if pixmaj:
            # [n_dma, P, dma_chunk]: each chunk contiguous in DRAM
            shard = np.ascontiguousarray(
                shard.reshape(P, npix // dma_chunk, dma_chunk).transpose(1, 0, 2)
            )
        else:
            shard = np.ascontiguousarray(shard)
        in_maps.append(
            {
                "bf": shard,
                "mask": mask_flat[b].reshape(npix // blk, blk),
                "min_std": minstd_flat[c0 : c0 + rows_per_core].reshape(P, 1),
                "sels": sels,
            }
        )
    return in_maps


def make_sels(nsel: int = 4, keep_dt: str = "bf16") -> np.ndarray:
    import ml_dtypes

    np_dt = {"bf16": ml_dtypes.bfloat16, "fp8": mybir.dt.np(mybir.dt.float8e4)}[
        keep_dt
    ]
    sels = np.zeros((nsel, nsel, P), dtype=np_dt)
    for k in range(nsel):
        sels[k, k, :] = 1.0
    return sels


def kernel(bf: np.ndarray, aspp_mask: np.ndarray, min_std: np.ndarray, **run_kwargs):
    bf = np.asarray(bf, dtype=np.float32)
    aspp_mask = np.asarray(aspp_mask, dtype=np.float32)
    min_std = np.asarray(min_std, dtype=np.float32)
    B, C, H, W = bf.shape
    npix = H * W

    nc = _get_nc(npix, dma_chunk=2048, bf_bufs=16, in_place=True, newton=0)
    in_maps = make_in_maps(bf, aspp_mask, min_std)
    res = run_bass_kernel_spmd(nc, in_maps, list(range(N_CORES)), **run_kwargs)

    out = np.empty((B, C), dtype=np.float32)
    rows_per_core = (B * C) // N_CORES
    cores_per_batch = C // rows_per_core
    for k in range(N_CORES):
        b = k // cores_per_batch
        c0 = (k % cores_per_batch) * rows_per_core
        out[b, c0 : c0 + rows_per_core] = res.results[k]["out"].reshape(rows_per_core)
    if run_kwargs:
        return out.reshape(B, C, 1, 1), res
    return out.reshape(B, C, 1, 1)



# revision 45
# speedup vs baseline: 1.0322x; 1.0303x over previous
"""Trainium2 Bass kernel for nn_BackgroundStd2D.

Computes, per (batch, channel): the unbiased std of bf over the pixels where
aspp_mask <= 0.5, clamped below by min_std + 1e-6.

Sharding: pure data parallel over the 1024 (batch, channel) rows of
bf.reshape(B*C, H*W); each of the 8 cores handles 128 rows (one batch's
half of channels) plus that batch's mask.

Per-core algorithm (rows on partitions, pixels on the free axis):
  keep128[p, f] = (mask[p*512+f] <= 0.5) in bf16 (exact 0/1)  [128, 512]
  keep is re-laid out to [4, n_chunks, 512] via a DRAM bounce; per 2048-px
  chunk the PE broadcasts the 4 keep rows across all 128 partitions into
  PSUM with one-hot bf16 selector matmuls (bf16 keeps PE at full rate).
  DVE scalar_tensor_tensor (stock ISA): bfk = (bf*1)*keep written in place
  over the bf tile, accum_out = s_part (fused multiply+sum, one pass).
  ACT activation(Square, accum_out): q_part = sum(bfk^2), second pass.
  n = sum(keep) via free-axis reduce + DRAM-bounce partition fold.
  Final [128,1] math: std = sqrt(q/(n-1) - s^2/(n(n-1))), out = max(std,
  min_std + 1e-6). All n-only terms (reciprocals, clamp floor) are emitted
  BEFORE the chunk loop so the in-order engine queues run them during the
  stream; ACT sqrt alone is accurate to ~4e-7 rel here, so no Newton steps.

  Steady state is HBM-bound: ~99-101us/pass vs ~96-98us pure-DMA floor
  (32 MiB/core at ~350 GB/s, 97% of the 358 GB/s HBM-per-NC limit; the
  floor drifts ~1-2us with ambient load on the shared device). The ~3.3us
  gap over the floor is per-consumer-stage interference (PE/DVE/ACT each
  add ~1us/pass) and is invariant to operand bytes (bf16/fp8 variants),
  PSUM-residency, issue rings, chunk size (2048 px optimal both ways),
  and buffer depth - measured exhaustively; see sweep.py/duel.py.
"""

import sys

sys.path.insert(0, "/opt/trn_rl_repo")

import numpy as np

import concourse.bass as bass
import concourse.tile as tile
from concourse import bacc, mybir
from concourse.bass_utils import run_bass_kernel_spmd

P = 128
N_CORES = 8
MIN_STD_VAL = 1e-05

F32 = mybir.dt.float32
BF16 = mybir.dt.bfloat16
ALU = mybir.AluOpType
ACTF = mybir.ActivationFunctionType


def build_bass(
    npix: int,
    dma_chunk: int = 4096,
    ttr_chunk: int = 2048,
    iters: int = 1,
    mode: str = "full",  # full | noact | nostt | dmaonly | dmapure
    rings: str = "sync",  # sync | dual | tri
    bf_bufs: int = 3,
    hw_loop: bool = False,
    dual_ring: bool = False,
    in_place: bool = False,
    pixmaj: bool = False,
    psum_bfk: bool = False,
    newton: int = 2,
    kp_bufs: int = 2,
    blk: int = 512,
    bfk_dt: str = "f32",
    bfk_bufs: int | None = None,
    dve_q: bool = False,
    keep_dt: str = "bf16",
    q_eng: str = "act",  # act | dve | gp
    wide_act: bool = False,  # square 2 dma chunks per ACT op
    unroll: int = 1,  # full passes per For_i iteration (bench: slope/unroll)
) -> bass.Bass:
    assert npix % dma_chunk == 0 and dma_chunk % ttr_chunk == 0
    assert ttr_chunk % blk == 0
    n_blk = npix // blk  # blk-pixel blocks; one keep row per block
    assert n_blk <= P
    n_dma = npix // dma_chunk
    n_ttr = npix // ttr_chunk
    BFK = {"f32": F32, "bf16": BF16}[bfk_dt]
    KDT = {"bf16": BF16, "fp8": mybir.dt.float8e4}[keep_dt]
    if bfk_bufs is None:
        bfk_bufs = bf_bufs

    nc = bacc.Bacc("TRN2", target_bir_lowering=False, debug=False)

    if pixmaj:
        bf_d = nc.dram_tensor(
            "bf", [npix // dma_chunk, P, dma_chunk], F32, kind="ExternalInput"
        ).ap()
    else:
        bf_d = nc.dram_tensor("bf", [P, npix], F32, kind="ExternalInput").ap()
    mask_d = nc.dram_tensor("mask", [n_blk, blk], F32, kind="ExternalInput").ap()
    mins_d = nc.dram_tensor("min_std", [P, 1], F32, kind="ExternalInput").ap()
    out_d = nc.dram_tensor("out", [P, 1], F32, kind="ExternalOutput").ap()
    keep_scratch = nc.dram_tensor("keep_scratch", [npix], KDT).ap()
    cnt_scratch = nc.dram_tensor("cnt_scratch", [P], F32).ap()
    n_scratch = nc.dram_tensor("n_scratch", [1], F32).ap()
    nsel = ttr_chunk // blk
    sel_d = nc.dram_tensor("sels", [nsel, nsel, P], KDT, kind="ExternalInput").ap()

    with tile.TileContext(nc) as tc:
        with (
            tc.tile_pool(name="singles", bufs=1) as singles,
            tc.tile_pool(name="bfp", bufs=bf_bufs) as bf_pool,
            tc.tile_pool(name="bfkp", bufs=bfk_bufs) as bfk_pool,
            tc.tile_pool(name="kps", bufs=kp_bufs, space="PSUM") as kp_pool,
            tc.tile_pool(name="fin", bufs=2) as fin,
        ):
            # One-hot row selectors: sel[k].T @ keep_r[:, J, :] broadcasts
            # keep row k across all 128 output partitions.
            sel_t = singles.tile([nsel, nsel, P], KDT)
            nc.scalar.dma_start(out=sel_t, in_=sel_d)
            sels = [sel_t[:, k, :] for k in range(nsel)]

            mask128 = singles.tile([n_blk, blk], F32)
            nc.scalar.dma_start(out=mask128, in_=mask_d)
            # keep is exactly 0/1 so bf16 is lossless; bf16 operands keep the
            # PE broadcast matmuls at full (non-fp32) rate.
            keep128 = singles.tile([n_blk, blk], KDT)
            nc.vector.tensor_scalar(
                out=keep128, in0=mask128, scalar1=0.5, scalar2=None, op0=ALU.is_le
            )
            # Bounce through DRAM to land keep in [4, n_ttr, 512] layout:
            # partition a holds pixel blocks {4m + a}.
            nc.scalar.dma_start(out=keep_scratch, in_=keep128)
            keep_r = singles.tile([nsel, n_ttr, blk], KDT)
            nc.scalar.dma_start(
                out=keep_r,
                in_=keep_scratch.rearrange("(m a f) -> a m f", a=nsel, f=blk),
            )

            # n = sum(keep): free-axis reduce, then fold the 128 partition
            # partials onto one partition via a DRAM bounce, reduce, and
            # broadcast the scalar back to all partitions.
            cnt = singles.tile([P, 1], F32)
            nc.vector.memset(cnt, 0.0)
            nc.vector.reduce_sum(
                out=cnt[0:n_blk, :], in_=keep128, axis=mybir.AxisListType.X
            )
            nc.scalar.dma_start(out=cnt_scratch, in_=cnt)
            cnt_row = singles.tile([1, P], F32)
            nc.scalar.dma_start(out=cnt_row, in_=cnt_scratch)
            n_scalar = singles.tile([1, 1], F32)
            nc.vector.reduce_sum(out=n_scalar, in_=cnt_row, axis=mybir.AxisListType.X)
            nc.scalar.dma_start(out=n_scratch, in_=n_scalar)
            n_b = singles.tile([P, 1], F32)
            nc.scalar.dma_start(out=n_b, in_=n_scratch.to_broadcast([P, 1]))

            minstd_sb = singles.tile([P, 1], F32)
            nc.scalar.dma_start(out=minstd_sb, in_=mins_d)

            # n-only final-math terms, emitted BEFORE the loop: engine queues
            # are in-order, so anything emitted after the loop only runs once
            # the last chunk drains. var = q/(n-1) - s^2*(1/(n(n-1))).
            inv_n = singles.tile([P, 1], F32)
            nc.vector.reciprocal(inv_n, n_b)
            nm1 = singles.tile([P, 1], F32)
            nc.vector.tensor_scalar_add(nm1, n_b, -1.0)
            inv_nm1 = singles.tile([P, 1], F32)
            nc.vector.reciprocal(inv_nm1, nm1)
            c2 = singles.tile([P, 1], F32)
            nc.vector.tensor_mul(c2, inv_n, inv_nm1)
            lower = singles.tile([P, 1], F32)
            nc.vector.tensor_scalar_add(lower, minstd_sb, MIN_STD_VAL / 10.0)

            s_parts = singles.tile([P, n_ttr], F32)
            q_parts = singles.tile(
                [P, n_ttr if psum_bfk else (n_dma // 2 if wide_act else n_dma)], F32
            )
            if mode != "full":
                nc.vector.memset(q_parts, 1.0)
                nc.vector.memset(s_parts, 1.0)

            import contextlib

            loop_cm = (
                tc.For_i(0, iters, 1) if hw_loop else contextlib.nullcontext(range(iters))
            )
            with loop_cm as _loop:
              for _it in range(1 if hw_loop else iters):
               for cc in range(unroll * n_dma):
                c = cc % n_dma
                bf_t = bf_pool.tile([P, dma_chunk], F32)
                if dual_ring or rings == "dual":
                    dma_eng = (nc.sync, nc.scalar)[c % 2]
                elif rings == "tri":
                    dma_eng = (nc.sync, nc.scalar, nc.gpsimd)[c % 3]
                else:
                    dma_eng = nc.sync
                bf_src = bf_d[c] if pixmaj else bf_d[:, c * dma_chunk : (c + 1) * dma_chunk]
                dma_eng.dma_start(out=bf_t, in_=bf_src)
                if mode == "dmapure":
                    continue
                if wide_act:
                    if c % 2 == 0:
                        bfk_wide = bfk_pool.tile([P, 2 * dma_chunk], BFK)
                    bfk_t = bfk_wide[:, (c % 2) * dma_chunk : (c % 2 + 1) * dma_chunk]
                else:
                    bfk_t = (
                        bf_t
                        if (in_place or psum_bfk)
                        else bfk_pool.tile([P, dma_chunk], BFK)
                    )
                if mode == "dmaonly":
                    nc.vector.reduce_sum(
                        out=s_parts[:, c : c + 1],
                        in_=bf_t[:, 0:8],
                        axis=mybir.AxisListType.X,
                    )
                    continue
                for h in range(dma_chunk // ttr_chunk):
                    j = c * (dma_chunk // ttr_chunk) + h
                    kp = kp_pool.tile([P, ttr_chunk], F32)
                    for k in range(ttr_chunk // blk):
                        nc.tensor.matmul(
                            kp[:, blk * k : blk * (k + 1)],
                            sels[k],
                            keep_r[:, j, :],
                            start=True,
                            stop=True,
                        )
                    if mode == "nostt":
                        nc.vector.reduce_sum(
                            out=s_parts[:, j : j + 1],
                            in_=kp[:, 0:8],
                            axis=mybir.AxisListType.X,
                        )
                    elif psum_bfk:
                        # bfk lives in PSUM, in place over kp: SBUF sees only
                        # the bf DMA write + the DVE read of bf.
                        nc.vector.scalar_tensor_tensor(
                            out=kp,
                            in0=bf_t[:, h * ttr_chunk : (h + 1) * ttr_chunk],
                            scalar=1.0,
                            in1=kp,
                            op0=ALU.mult,
                            op1=ALU.mult,
                            accum_out=s_parts[:, j : j + 1],
                        )
                        if mode == "full":
                            nc.scalar.activation(
                                out=kp,
                                in_=kp,
                                func=ACTF.Square,
                                accum_out=q_parts[:, j : j + 1],
                            )
                    else:
                        nc.vector.scalar_tensor_tensor(
                            out=bfk_t[:, h * ttr_chunk : (h + 1) * ttr_chunk],
                            in0=bf_t[:, h * ttr_chunk : (h + 1) * ttr_chunk],
                            scalar=1.0,
                            in1=kp,
                            op0=ALU.mult,
                            op1=ALU.mult,
                            accum_out=s_parts[:, j : j + 1],
                        )
                if mode == "full" and wide_act:
                    if c % 2 == 1:
                        nc.scalar.activation(
                            out=bfk_wide,
                            in_=bfk_wide,
                            func=ACTF.Square,
                            accum_out=q_parts[:, c // 2 : c // 2 + 1],
                        )
                elif mode == "full" and not psum_bfk:
                    if dve_q or q_eng == "dve":
                        nc.vector.scalar_tensor_tensor(
                            out=bfk_t,
                            in0=bfk_t,
                            scalar=1.0,
                            in1=bfk_t,
                            op0=ALU.mult,
                            op1=ALU.mult,
                            accum_out=q_parts[:, c : c + 1],
                        )
                    elif q_eng == "gp":
                        nc.gpsimd.scalar_tensor_tensor(
                            out=bfk_t,
                            in0=bfk_t,
                            scalar=1.0,
                            in1=bfk_t,
                            op0=ALU.mult,
                            op1=ALU.mult,
                            accum_out=q_parts[:, c : c + 1],
                        )
                    else:
                        nc.scalar.activation(
                            out=bfk_t,
                            in_=bfk_t,
                            func=ACTF.Square,
                            accum_out=q_parts[:, c : c + 1],
                        )

            s = fin.tile([P, 1], F32)
            nc.vector.reduce_sum(out=s, in_=s_parts, axis=mybir.AxisListType.X)
            q = fin.tile([P, 1], F32)
            nc.vector.reduce_sum(out=q, in_=q_parts, axis=mybir.AxisListType.X)

            t1 = fin.tile([P, 1], F32)
            nc.vector.scalar_tensor_tensor(
                out=t1, in0=s, scalar=c2, in1=s, op0=ALU.mult, op1=ALU.mult
            )
            var = fin.tile([P, 1], F32)
            nc.vector.scalar_tensor_tensor(
                out=var, in0=q, scalar=inv_nm1, in1=t1, op0=ALU.mult, op1=ALU.subtract
            )

            std = fin.tile([P, 1], F32)
            nc.scalar.sqrt(std, var)
            # ACT sqrt has a loose ULP budget; Newton steps pin it to f32.
            for it in range(newton):
                r = fin.tile([P, 1], F32, name=f"r{it}")
                nc.vector.reciprocal(r, std)
                t = fin.tile([P, 1], F32, name=f"t{it}")
                nc.vector.tensor_mul(t, var, r)
                u = fin.tile([P, 1], F32, name=f"u{it}")
                nc.vector.tensor_add(u, std, t)
                std = fin.tile([P, 1], F32, name=f"std{it}")
                nc.vector.tensor_scalar_mul(std, u, 0.5)

            outv = fin.tile([P, 1], F32)
            nc.vector.tensor_max(outv, std, lower)
            nc.sync.dma_start(out=out_d, in_=outv)

    nc.compile()
    return nc


_NC_CACHE: dict[tuple, bass.Bass] = {}


def _get_nc(npix: int, **kwargs) -> bass.Bass:
    key = (npix, tuple(sorted(kwargs.items())))
    if key not in _NC_CACHE:
        _NC_CACHE[key] = build_bass(npix, **kwargs)
    return _NC_CACHE[key]


def make_in_maps(
    bf: np.ndarray,
    aspp_mask: np.ndarray,
    min_std: np.ndarray,
    pixmaj: bool = False,
    dma_chunk: int = 4096,
    ttr_chunk: int = 2048,
    blk: int = 512,
    keep_dt: str = "bf16",
):
    B, C, H, W = bf.shape
    npix = H * W
    rows = bf.reshape(B * C, npix)
    rows_per_core = (B * C) // N_CORES
    cores_per_batch = C // rows_per_core
    mask_flat = np.ascontiguousarray(aspp_mask.reshape(B, npix))
    minstd_flat = np.ascontiguousarray(min_std.reshape(C))
    sels = make_sels(ttr_chunk // blk, keep_dt)
    in_maps = []
    for k in range(N_CORES):
        b = k // cores_per_batch
        c0 = (k % cores_per_batch) * rows_per_core
        shard = rows[k * rows_per_core : (k + 1) * rows_per_core]
        if pixmaj:
            # [n_dma, P, dma_chunk]: each chunk contiguous in DRAM
            shard = np.ascontiguousarray(
                shard.reshape(P, npix // dma_chunk, dma_chunk).transpose(1, 0, 2)
            )
        else:
            shard = np.ascontiguousarray(shard)
        in_maps.append(
            {
                "bf": shard,
                "mask": mask_flat[b].reshape(npix // blk, blk),
                "min_std": minstd_flat[c0 : c0 + rows_per_core].reshape(P, 1),
                "sels": sels,
            }
        )
    return in_maps


def make_sels(nsel: int = 4, keep_dt: str = "bf16") -> np.ndarray:
    import ml_dtypes

    np_dt = {"bf16": ml_dtypes.bfloat16, "fp8": mybir.dt.np(mybir.dt.float8e4)}[
        keep_dt
    ]
    sels = np.zeros((nsel, nsel, P), dtype=np_dt)
    for k in range(nsel):
        sels[k, k, :] = 1.0
    return sels


def kernel(bf: np.ndarray, aspp_mask: np.ndarray, min_std: np.ndarray, **run_kwargs):
    bf = np.asarray(bf, dtype=np.float32)
    aspp_mask = np.asarray(aspp_mask, dtype=np.float32)
    min_std = np.asarray(min_std, dtype=np.float32)
    B, C, H, W = bf.shape
    npix = H * W

    nc = _get_nc(npix, dma_chunk=2048, bf_bufs=16, in_place=True, newton=0)
    in_maps = make_in_maps(bf, aspp_mask, min_std)
    res = run_bass_kernel_spmd(nc, in_maps, list(range(N_CORES)), **run_kwargs)

    out = np.empty((B, C), dtype=np.float32)
    rows_per_core = (B * C) // N_CORES
    cores_per_batch = C // rows_per_core
    for k in range(N_CORES):
        b = k // cores_per_batch
        c0 = (k % cores_per_batch) * rows_per_core
        out[b, c0 : c0 + rows_per_core] = res.results[k]["out"].reshape(rows_per_core)
    if run_kwargs:
        return out.reshape(B, C, 1, 1), res
    return out.reshape(B, C, 1, 1)



# revision 46
# speedup vs baseline: 1.0395x; 1.0071x over previous
"""Trainium2 Bass kernel for nn_BackgroundStd2D.

Computes, per (batch, channel): the unbiased std of bf over the pixels where
aspp_mask <= 0.5, clamped below by min_std + 1e-6.

Sharding: pure data parallel over the 1024 (batch, channel) rows of
bf.reshape(B*C, H*W); each of the 8 cores handles 128 rows (one batch's
half of channels) plus that batch's mask.

Per-core algorithm (rows on partitions, pixels on the free axis):
  keep128[p, f] = (mask[p*512+f] <= 0.5) in bf16 (exact 0/1)  [128, 512]
  keep is re-laid out to [4, n_chunks, 512] via a DRAM bounce; per 2048-px
  chunk the PE broadcasts the 4 keep rows across all 128 partitions into
  PSUM with one-hot bf16 selector matmuls (bf16 keeps PE at full rate).
  DVE scalar_tensor_tensor (stock ISA): bfk = (bf*1)*keep written in place
  over the bf tile, accum_out = s_part (fused multiply+sum, one pass).
  ACT activation(Square, accum_out): q_part = sum(bfk^2), second pass.
  n = sum(keep) via free-axis reduce + DRAM-bounce partition fold.
  Final [128,1] math: std = sqrt(q/(n-1) - s^2/(n(n-1))), out = max(std,
  min_std + 1e-6). All n-only terms (reciprocals, clamp floor) are emitted
  BEFORE the chunk loop so the in-order engine queues run them during the
  stream; ACT sqrt alone is accurate to ~4e-7 rel here, so no Newton steps.

  Steady state is HBM-bound at ~96-97us/pass, the pure-DMA floor (32
  MiB/core at ~350 GB/s, 97% of the 358 GB/s HBM-per-NC limit; drifts
  ~1-2us with ambient load on the shared device). A For_i-wrapped bench
  iteration additionally pays a ~4-5us loop-boundary pipeline drain, so
  slope benches must unroll several passes per iteration (test.py uses
  UNROLL=4) or they overstate per-pass time. Engine work (PE/DVE/ACT) is
  fully hidden behind the DMA stream; chunk size 2048 px is optimal in
  both directions and all dtype/ring/PSUM/buffer variants measured equal
  or worse - see sweep.py/duel.py.
"""

import sys

sys.path.insert(0, "/opt/trn_rl_repo")

import numpy as np

import concourse.bass as bass
import concourse.tile as tile
from concourse import bacc, mybir
from concourse.bass_utils import run_bass_kernel_spmd

P = 128
N_CORES = 8
MIN_STD_VAL = 1e-05

F32 = mybir.dt.float32
BF16 = mybir.dt.bfloat16
ALU = mybir.AluOpType
ACTF = mybir.ActivationFunctionType


def build_bass(
    npix: int,
    dma_chunk: int = 4096,
    ttr_chunk: int = 2048,
    iters: int = 1,
    mode: str = "full",  # full | noact | nostt | dmaonly | dmapure
    rings: str = "sync",  # sync | dual | tri
    bf_bufs: int = 3,
    hw_loop: bool = False,
    dual_ring: bool = False,
    in_place: bool = False,
    pixmaj: bool = False,
    psum_bfk: bool = False,
    newton: int = 2,
    kp_bufs: int = 2,
    blk: int = 512,
    bfk_dt: str = "f32",
    bfk_bufs: int | None = None,
    dve_q: bool = False,
    keep_dt: str = "bf16",
    q_eng: str = "act",  # act | dve | gp
    wide_act: bool = False,  # square 2 dma chunks per ACT op
    unroll: int = 1,  # full passes per For_i iteration (bench: slope/unroll)
) -> bass.Bass:
    assert npix % dma_chunk == 0 and dma_chunk % ttr_chunk == 0
    assert ttr_chunk % blk == 0
    n_blk = npix // blk  # blk-pixel blocks; one keep row per block
    assert n_blk <= P
    n_dma = npix // dma_chunk
    n_ttr = npix // ttr_chunk
    BFK = {"f32": F32, "bf16": BF16}[bfk_dt]
    KDT = {"bf16": BF16, "fp8": mybir.dt.float8e4}[keep_dt]
    if bfk_bufs is None:
        bfk_bufs = bf_bufs

    nc = bacc.Bacc("TRN2", target_bir_lowering=False, debug=False)

    if pixmaj:
        bf_d = nc.dram_tensor(
            "bf", [npix // dma_chunk, P, dma_chunk], F32, kind="ExternalInput"
        ).ap()
    else:
        bf_d = nc.dram_tensor("bf", [P, npix], F32, kind="ExternalInput").ap()
    mask_d = nc.dram_tensor("mask", [n_blk, blk], F32, kind="ExternalInput").ap()
    mins_d = nc.dram_tensor("min_std", [P, 1], F32, kind="ExternalInput").ap()
    out_d = nc.dram_tensor("out", [P, 1], F32, kind="ExternalOutput").ap()
    keep_scratch = nc.dram_tensor("keep_scratch", [npix], KDT).ap()
    cnt_scratch = nc.dram_tensor("cnt_scratch", [P], F32).ap()
    n_scratch = nc.dram_tensor("n_scratch", [1], F32).ap()
    nsel = ttr_chunk // blk
    sel_d = nc.dram_tensor("sels", [nsel, nsel, P], KDT, kind="ExternalInput").ap()

    with tile.TileContext(nc) as tc:
        with (
            tc.tile_pool(name="singles", bufs=1) as singles,
            tc.tile_pool(name="bfp", bufs=bf_bufs) as bf_pool,
            tc.tile_pool(name="bfkp", bufs=bfk_bufs) as bfk_pool,
            tc.tile_pool(name="kps", bufs=kp_bufs, space="PSUM") as kp_pool,
            tc.tile_pool(name="fin", bufs=2) as fin,
        ):
            # One-hot row selectors: sel[k].T @ keep_r[:, J, :] broadcasts
            # keep row k across all 128 output partitions.
            sel_t = singles.tile([nsel, nsel, P], KDT)
            nc.scalar.dma_start(out=sel_t, in_=sel_d)
            sels = [sel_t[:, k, :] for k in range(nsel)]

            mask128 = singles.tile([n_blk, blk], F32)
            nc.scalar.dma_start(out=mask128, in_=mask_d)
            # keep is exactly 0/1 so bf16 is lossless; bf16 operands keep the
            # PE broadcast matmuls at full (non-fp32) rate.
            keep128 = singles.tile([n_blk, blk], KDT)
            nc.vector.tensor_scalar(
                out=keep128, in0=mask128, scalar1=0.5, scalar2=None, op0=ALU.is_le
            )
            # Bounce through DRAM to land keep in [4, n_ttr, 512] layout:
            # partition a holds pixel blocks {4m + a}.
            nc.scalar.dma_start(out=keep_scratch, in_=keep128)
            keep_r = singles.tile([nsel, n_ttr, blk], KDT)
            nc.scalar.dma_start(
                out=keep_r,
                in_=keep_scratch.rearrange("(m a f) -> a m f", a=nsel, f=blk),
            )

            # n = sum(keep): free-axis reduce, then fold the 128 partition
            # partials onto one partition via a DRAM bounce, reduce, and
            # broadcast the scalar back to all partitions.
            cnt = singles.tile([P, 1], F32)
            nc.vector.memset(cnt, 0.0)
            nc.vector.reduce_sum(
                out=cnt[0:n_blk, :], in_=keep128, axis=mybir.AxisListType.X
            )
            nc.scalar.dma_start(out=cnt_scratch, in_=cnt)
            cnt_row = singles.tile([1, P], F32)
            nc.scalar.dma_start(out=cnt_row, in_=cnt_scratch)
            n_scalar = singles.tile([1, 1], F32)
            nc.vector.reduce_sum(out=n_scalar, in_=cnt_row, axis=mybir.AxisListType.X)
            nc.scalar.dma_start(out=n_scratch, in_=n_scalar)
            n_b = singles.tile([P, 1], F32)
            nc.scalar.dma_start(out=n_b, in_=n_scratch.to_broadcast([P, 1]))

            minstd_sb = singles.tile([P, 1], F32)
            nc.scalar.dma_start(out=minstd_sb, in_=mins_d)

            # n-only final-math terms, emitted BEFORE the loop: engine queues
            # are in-order, so anything emitted after the loop only runs once
            # the last chunk drains. var = q/(n-1) - s^2*(1/(n(n-1))).
            inv_n = singles.tile([P, 1], F32)
            nc.vector.reciprocal(inv_n, n_b)
            nm1 = singles.tile([P, 1], F32)
            nc.vector.tensor_scalar_add(nm1, n_b, -1.0)
            inv_nm1 = singles.tile([P, 1], F32)
            nc.vector.reciprocal(inv_nm1, nm1)
            c2 = singles.tile([P, 1], F32)
            nc.vector.tensor_mul(c2, inv_n, inv_nm1)
            lower = singles.tile([P, 1], F32)
            nc.vector.tensor_scalar_add(lower, minstd_sb, MIN_STD_VAL / 10.0)

            s_parts = singles.tile([P, n_ttr], F32)
            q_parts = singles.tile(
                [P, n_ttr if psum_bfk else (n_dma // 2 if wide_act else n_dma)], F32
            )
            if mode != "full":
                nc.vector.memset(q_parts, 1.0)
                nc.vector.memset(s_parts, 1.0)

            import contextlib

            loop_cm = (
                tc.For_i(0, iters, 1) if hw_loop else contextlib.nullcontext(range(iters))
            )
            with loop_cm as _loop:
              for _it in range(1 if hw_loop else iters):
               for cc in range(unroll * n_dma):
                c = cc % n_dma
                bf_t = bf_pool.tile([P, dma_chunk], F32)
                if dual_ring or rings == "dual":
                    dma_eng = (nc.sync, nc.scalar)[c % 2]
                elif rings == "tri":
                    dma_eng = (nc.sync, nc.scalar, nc.gpsimd)[c % 3]
                else:
                    dma_eng = nc.sync
                bf_src = bf_d[c] if pixmaj else bf_d[:, c * dma_chunk : (c + 1) * dma_chunk]
                dma_eng.dma_start(out=bf_t, in_=bf_src)
                if mode == "dmapure":
                    continue
                if wide_act:
                    if c % 2 == 0:
                        bfk_wide = bfk_pool.tile([P, 2 * dma_chunk], BFK)
                    bfk_t = bfk_wide[:, (c % 2) * dma_chunk : (c % 2 + 1) * dma_chunk]
                else:
                    bfk_t = (
                        bf_t
                        if (in_place or psum_bfk)
                        else bfk_pool.tile([P, dma_chunk], BFK)
                    )
                if mode == "dmaonly":
                    nc.vector.reduce_sum(
                        out=s_parts[:, c : c + 1],
                        in_=bf_t[:, 0:8],
                        axis=mybir.AxisListType.X,
                    )
                    continue
                for h in range(dma_chunk // ttr_chunk):
                    j = c * (dma_chunk // ttr_chunk) + h
                    kp = kp_pool.tile([P, ttr_chunk], F32)
                    for k in range(ttr_chunk // blk):
                        nc.tensor.matmul(
                            kp[:, blk * k : blk * (k + 1)],
                            sels[k],
                            keep_r[:, j, :],
                            start=True,
                            stop=True,
                        )
                    if mode == "nostt":
                        nc.vector.reduce_sum(
                            out=s_parts[:, j : j + 1],
                            in_=kp[:, 0:8],
                            axis=mybir.AxisListType.X,
                        )
                    elif psum_bfk:
                        # bfk lives in PSUM, in place over kp: SBUF sees only
                        # the bf DMA write + the DVE read of bf.
                        nc.vector.scalar_tensor_tensor(
                            out=kp,
                            in0=bf_t[:, h * ttr_chunk : (h + 1) * ttr_chunk],
                            scalar=1.0,
                            in1=kp,
                            op0=ALU.mult,
                            op1=ALU.mult,
                            accum_out=s_parts[:, j : j + 1],
                        )
                        if mode == "full":
                            nc.scalar.activation(
                                out=kp,
                                in_=kp,
                                func=ACTF.Square,
                                accum_out=q_parts[:, j : j + 1],
                            )
                    else:
                        nc.vector.scalar_tensor_tensor(
                            out=bfk_t[:, h * ttr_chunk : (h + 1) * ttr_chunk],
                            in0=bf_t[:, h * ttr_chunk : (h + 1) * ttr_chunk],
                            scalar=1.0,
                            in1=kp,
                            op0=ALU.mult,
                            op1=ALU.mult,
                            accum_out=s_parts[:, j : j + 1],
                        )
                if mode == "full" and wide_act:
                    if c % 2 == 1:
                        nc.scalar.activation(
                            out=bfk_wide,
                            in_=bfk_wide,
                            func=ACTF.Square,
                            accum_out=q_parts[:, c // 2 : c // 2 + 1],
                        )
                elif mode == "full" and not psum_bfk:
                    if dve_q or q_eng == "dve":
                        nc.vector.scalar_tensor_tensor(
                            out=bfk_t,
                            in0=bfk_t,
                            scalar=1.0,
                            in1=bfk_t,
                            op0=ALU.mult,
                            op1=ALU.mult,
                            accum_out=q_parts[:, c : c + 1],
                        )
                    elif q_eng == "gp":
                        nc.gpsimd.scalar_tensor_tensor(
                            out=bfk_t,
                            in0=bfk_t,
                            scalar=1.0,
                            in1=bfk_t,
                            op0=ALU.mult,
                            op1=ALU.mult,
                            accum_out=q_parts[:, c : c + 1],
                        )
                    else:
                        nc.scalar.activation(
                            out=bfk_t,
                            in_=bfk_t,
                            func=ACTF.Square,
                            accum_out=q_parts[:, c : c + 1],
                        )

            s = fin.tile([P, 1], F32)
            nc.vector.reduce_sum(out=s, in_=s_parts, axis=mybir.AxisListType.X)
            q = fin.tile([P, 1], F32)
            nc.vector.reduce_sum(out=q, in_=q_parts, axis=mybir.AxisListType.X)

            t1 = fin.tile([P, 1], F32)
            nc.vector.scalar_tensor_tensor(
                out=t1, in0=s, scalar=c2, in1=s, op0=ALU.mult, op1=ALU.mult
            )
            var = fin.tile([P, 1], F32)
            nc.vector.scalar_tensor_tensor(
                out=var, in0=q, scalar=inv_nm1, in1=t1, op0=ALU.mult, op1=ALU.subtract
            )

            std = fin.tile([P, 1], F32)
            nc.scalar.sqrt(std, var)
            # ACT sqrt has a loose ULP budget; Newton steps pin it to f32.
            for it in range(newton):
                r = fin.tile([P, 1], F32, name=f"r{it}")
                nc.vector.reciprocal(r, std)
                t = fin.tile([P, 1], F32, name=f"t{it}")
                nc.vector.tensor_mul(t, var, r)
                u = fin.tile([P, 1], F32, name=f"u{it}")
                nc.vector.tensor_add(u, std, t)
                std = fin.tile([P, 1], F32, name=f"std{it}")
                nc.vector.tensor_scalar_mul(std, u, 0.5)

            outv = fin.tile([P, 1], F32)
            nc.vector.tensor_max(outv, std, lower)
            nc.sync.dma_start(out=out_d, in_=outv)

    nc.compile()
    return nc


_NC_CACHE: dict[tuple, bass.Bass] = {}


def _get_nc(npix: int, **kwargs) -> bass.Bass:
    key = (npix, tuple(sorted(kwargs.items())))
    if key not in _NC_CACHE:
        _NC_CACHE[key] = build_bass(npix, **kwargs)
    return _NC_CACHE[key]


def make_in_maps(
    bf: np.ndarray,
    aspp_mask: np.ndarray,
    min_std: np.ndarray,
    pixmaj: bool = False,
    dma_chunk: int = 4096,
    ttr_chunk: int = 2048,
    blk: int = 512,
    keep_dt: str = "bf16",
):
    B, C, H, W = bf.shape
    npix = H * W
    rows = bf.reshape(B * C, npix)
    rows_per_core = (B * C) // N_CORES
    cores_per_batch = C // rows_per_core
    mask_flat = np.ascontiguousarray(aspp_mask.reshape(B, npix))
    minstd_flat = np.ascontiguousarray(min_std.reshape(C))
    sels = make_sels(ttr_chunk // blk, keep_dt)
    in_maps = []
    for k in range(N_CORES):
        b = k // cores_per_batch
        c0 = (k % cores_per_batch) * rows_per_core
        shard = rows[k * rows_per_core : (k + 1) * rows_per_core]
        if pixmaj:
            # [n_dma, P, dma_chunk]: each chunk contiguous in DRAM
            shard = np.ascontiguousarray(
                shard.reshape(P, npix // dma_chunk, dma_chunk).transpose(1, 0, 2)
            )
        else:
            shard = np.ascontiguousarray(shard)
        in_maps.append(
            {
                "bf": shard,
                "mask": mask_flat[b].reshape(npix // blk, blk),
                "min_std": minstd_flat[c0 : c0 + rows_per_core].reshape(P, 1),
                "sels": sels,
            }
        )
    return in_maps


def make_sels(nsel: int = 4, keep_dt: str = "bf16") -> np.ndarray:
    import ml_dtypes

    np_dt = {"bf16": ml_dtypes.bfloat16, "fp8": mybir.dt.np(mybir.dt.float8e4)}[
        keep_dt
    ]
    sels = np.zeros((nsel, nsel, P), dtype=np_dt)
    for k in range(nsel):
        sels[k, k, :] = 1.0
    return sels


def kernel(bf: np.ndarray, aspp_mask: np.ndarray, min_std: np.ndarray, **run_kwargs):
    bf = np.asarray(bf, dtype=np.float32)
    aspp_mask = np.asarray(aspp_mask, dtype=np.float32)
    min_std = np.asarray(min_std, dtype=np.float32)
    B, C, H, W = bf.shape
    npix = H * W

    nc = _get_nc(npix, dma_chunk=2048, bf_bufs=16, in_place=True, newton=0)
    in_maps = make_in_maps(bf, aspp_mask, min_std)
    res = run_bass_kernel_spmd(nc, in_maps, list(range(N_CORES)), **run_kwargs)

    out = np.empty((B, C), dtype=np.float32)
    rows_per_core = (B * C) // N_CORES
    cores_per_batch = C // rows_per_core
    for k in range(N_CORES):
        b = k // cores_per_batch
        c0 = (k % cores_per_batch) * rows_per_core
        out[b, c0 : c0 + rows_per_core] = res.results[k]["out"].reshape(rows_per_core)
    if run_kwargs:
        return out.reshape(B, C, 1, 1), res
    return out.reshape(B, C, 1, 1)

